# revision 26
# baseline (speedup 1.0000x reference)
"""Trainium2 Bass kernel for nn_Critic_QuadAdv_MultiheadAttention.

Self-contained: accepts FULL inputs (as produced by setup_inputs()), shards
across 8 NeuronCores (pure data parallel over batch), runs one fused Bass
kernel per core, gathers and returns the full output tuple
(multi_head_attention [65536,128], agent_attention [65536,128]).
"""

import sys

sys.path.insert(0, "/opt/trn_rl_repo")

import numpy as np
import ml_dtypes

import concourse.bass as bass
import concourse.tile as tile
from concourse import bacc, mybir
from concourse.bass_utils import run_bass_kernel_spmd

BF16 = ml_dtypes.bfloat16

# Model constants (hardcoded per spec)
NUM_HEADS = 8
ATTN_SIZE = 16
NUM_AGENTS = 8
NBR_OBS_DIM = 12
SELF_OBS_DIM = 18
NUM_ADV = 8
HID = 128
BATCH = 65536
NCORES = 8
NB_PER_CORE = BATCH // NCORES          # 8192 batch rows per core
BK = 256                               # batch rows per block
BJ = BK * NUM_ADV                      # 2048 mlp rows per block
N_BLOCKS_FULL = NB_PER_CORE // BK      # 32

_DT = mybir.dt
_AF = mybir.ActivationFunctionType
_OP = mybir.AluOpType


def _bc(ap: bass.AP, n: int, axis: int) -> bass.AP:
    """Insert a broadcast (step 0, count n) free dim at position `axis` of ap.ap."""
    new = list(ap.ap)
    new.insert(axis, [0, n])
    return bass.AP(tensor=ap.tensor, offset=ap.offset, ap=new)


def build_bass(n_blocks: int = N_BLOCKS_FULL):
    nc = bacc.Bacc(None, target_bir_lowering=False)
    NK = n_blocks * BK          # batch rows this core
    NG = NK // NUM_AGENTS       # groups this core
    NCOL = n_blocks * 512       # packed input cols

    inp_d = nc.dram_tensor("inp", [128, NCOL], _DT.bfloat16, kind="ExternalInput")
    w1e_d = nc.dram_tensor("w1e", [128, 128], _DT.bfloat16, kind="ExternalInput")
    w2e_d = nc.dram_tensor("w2e", [128, 128], _DT.bfloat16, kind="ExternalInput")
    w1v_d = nc.dram_tensor("w1v", [128, 128], _DT.bfloat16, kind="ExternalInput")
    w2v_d = nc.dram_tensor("w2v", [128, 128], _DT.bfloat16, kind="ExternalInput")
    w1a_d = nc.dram_tensor("w1a", [128, 128], _DT.bfloat16, kind="ExternalInput")
    w2a_d = nc.dram_tensor("w2a", [128, 128], _DT.bfloat16, kind="ExternalInput")
    bias_d = nc.dram_tensor("bias", [128, 6], _DT.float32, kind="ExternalInput")
    ones_d = nc.dram_tensor("ones", [128, 1], _DT.bfloat16, kind="ExternalInput")
    ihd_d = nc.dram_tensor("ihd", [128, 8], _DT.bfloat16, kind="ExternalInput")
    ident_d = nc.dram_tensor("ident", [128, 128], _DT.bfloat16, kind="ExternalInput")
    ihdT_d = nc.dram_tensor("ihdT", [8, 128], _DT.bfloat16, kind="ExternalInput")
    aa_d = nc.dram_tensor("aa", [128, NK], _DT.float32, kind="ExternalOutput")
    out2_d = nc.dram_tensor("out2", [128, NG], _DT.float32, kind="ExternalOutput")

    S1 = 1.0 / (NUM_ADV * float(np.sqrt(HID)))      # level-1: mean/8 and /sqrt(128)
    S2 = 1.0 / (NUM_AGENTS * float(np.sqrt(ATTN_SIZE)))  # level-2: mean/8 and /4

    with tile.TileContext(nc) as tc:
        with (
            tc.tile_pool(name="const", bufs=1) as constp,
            tc.tile_pool(name="io", bufs=4) as iop,
            tc.tile_pool(name="work", bufs=3) as work,
            tc.tile_pool(name="mlp_ps", bufs=2, space="PSUM") as mlp_ps,
            tc.tile_pool(name="sc_ps", bufs=2, space="PSUM") as sc_ps,
            tc.tile_pool(name="l2_ps", bufs=1, space="PSUM") as l2_ps,
            tc.tile_pool(name="dram", bufs=2, space="DRAM") as dram,
        ):
            w1e = constp.tile([128, 128], _DT.bfloat16)
            w2e = constp.tile([128, 128], _DT.bfloat16)
            w1v = constp.tile([128, 128], _DT.bfloat16)
            w2v = constp.tile([128, 128], _DT.bfloat16)
            w1a = constp.tile([128, 128], _DT.bfloat16)
            w2a = constp.tile([128, 128], _DT.bfloat16)
            biases = constp.tile([128, 6], _DT.float32)
            ones = constp.tile([128, 1], _DT.bfloat16)
            ihd = constp.tile([128, 8], _DT.bfloat16)
            ident = constp.tile([128, 128], _DT.bfloat16)
            ihdT = constp.tile([8, 128], _DT.bfloat16)
            nc.sync.dma_start(w1e[:], w1e_d[:])
            nc.sync.dma_start(w2e[:], w2e_d[:])
            nc.sync.dma_start(w1v[:], w1v_d[:])
            nc.sync.dma_start(w2v[:], w2v_d[:])
            nc.sync.dma_start(w1a[:], w1a_d[:])
            nc.sync.dma_start(w2a[:], w2a_d[:])
            nc.sync.dma_start(biases[:], bias_d[:])
            nc.sync.dma_start(ones[:], ones_d[:])
            nc.sync.dma_start(ihd[:], ihd_d[:])
            nc.sync.dma_start(ident[:], ident_d[:])
            nc.sync.dma_start(ihdT[:], ihdT_d[:])

            def mlp_layer(dst, src_fn, w, bias_col, first=False, xin=None):
                """One [128->128] layer over BJ cols: matmul halves + tanh."""
                for half in range(2):
                    ps = mlp_ps.tile([128, 1024], _DT.float32, tag="mlp")
                    for qq in range(2):
                        if first:
                            q = half * 2 + qq
                            nc.tensor.matmul(
                                ps[:, qq * 512:(qq + 1) * 512],
                                w[32 * q:32 * (q + 1), :],
                                xin[32 * q:32 * (q + 1), :],
                                tile_position=(32 * q, 0),
                            )
                        else:
                            s = half * 1024 + qq * 512
                            nc.tensor.matmul(
                                ps[:, qq * 512:(qq + 1) * 512],
                                w[:],
                                src_fn[:, s:s + 512],
                            )
                    nc.scalar.activation(
                        dst[:, half * 1024:(half + 1) * 1024],
                        ps[:],
                        _AF.Tanh,
                        bias=biases[:, bias_col:bias_col + 1],
                    )

            assert n_blocks % 4 == 0
            for sb in range(n_blocks // 4):
              aaf_s = work.tile([128, 1024], _DT.float32, tag="aafs")
              aab_s = work.tile([128, 1024], _DT.bfloat16, tag="aabs")
              for bi in range(4):
                blk = sb * 4 + bi
                xin = iop.tile([128, 512], _DT.bfloat16, tag="xin")
                nc.gpsimd.dma_start(xin[:], inp_d[:, blk * 512:(blk + 1) * 512])

                h1e = work.tile([128, BJ], _DT.bfloat16, tag="h1e")
                embT = work.tile([128, BJ], _DT.bfloat16, tag="embT")
                h1v = work.tile([128, BJ], _DT.bfloat16, tag="h1v")
                valT = work.tile([128, BJ], _DT.bfloat16, tag="valT")

                mlp_layer(h1e, None, w1e, 0, first=True, xin=xin)
                mlp_layer(embT, h1e, w2e, 1)
                mlp_layer(h1v, embT, w1v, 2)
                mlp_layer(valT, h1v, w2v, 3)

                # ---- level-1 attention (cols are n-major: j = n*256 + k) ----
                # q_sum[h,k] = sum_n emb[h, n*256+k]  (pair tree, contiguous halves)
                qs1 = work.tile([128, 1024], _DT.bfloat16, tag="qs1")
                nc.vector.tensor_add(qs1[:], embT[:, 0:1024], embT[:, 1024:2048])
                qs2 = work.tile([128, 512], _DT.bfloat16, tag="qs2")
                nc.vector.tensor_add(qs2[:], qs1[:, 0:512], qs1[:, 512:1024])
                qsum = work.tile([128, 256], _DT.bfloat16, tag="qsum")
                nc.vector.tensor_add(qsum[:], qs2[:, 0:256], qs2[:, 256:512])

                # prod[h, (n,k)] = emb[h,(n,k)] * q_sum[h,k]
                prod = work.tile([128, BJ], _DT.bfloat16, tag="prod")
                emb_nk = embT[:].rearrange("p (n k) -> p n k", n=8)
                prod_nk = prod[:].rearrange("p (n k) -> p n k", n=8)
                nc.vector.tensor_mul(prod_nk, emb_nk, _bc(qsum[:], 8, 1))

                # scores via prod-as-weights: psum_sc[p, t] = sum_h prod[h, t*128+p]
                # col j = t*128+p  ->  n = t//2, khi = t%2, klo = p
                # col order: (khi, n): ssp[:, khi*8+n] = scores for k=khi*128+klo
                ssp = sc_ps.tile([128, 16], _DT.float32, tag="sc")
                for t in range(16):
                    n_, khi_ = t // 2, t % 2
                    c = khi_ * 8 + n_
                    nc.tensor.matmul(
                        ssp[:, c:c + 1],
                        prod[:, t * 128:(t + 1) * 128],
                        ones[:],
                    )
                # exp with fused scale -> esc[klo, (khi,n)]
                esc = work.tile([128, 16], _DT.float32, tag="esc")
                nc.scalar.activation(esc[:], ssp[:], _AF.Exp, scale=S1)
                # denom over n per khi
                den = work.tile([128, 2], _DT.float32, tag="den")
                esc_kn = esc[:].rearrange("p (khi n) -> p khi n", khi=2)
                nc.vector.tensor_reduce(den[:], esc_kn, axis=mybir.AxisListType.X,
                                        op=_OP.add)
                rec = work.tile([128, 2], _DT.float32, tag="rec")
                nc.vector.reciprocal(rec[:], den[:])
                # attn[klo, (khi,n)] = esc * rec[khi]  (128-col pad: xbar
                # needs free dim to be a multiple of 128)
                attn_pad = work.tile([128, 128], _DT.bfloat16, tag="attn")
                attn = attn_pad[:, 0:16]
                nc.vector.memset(attn_pad[:, 16:128], 0.0)
                for khi in range(2):
                    nc.vector.tensor_scalar_mul(
                        attn[:, khi * 8:(khi + 1) * 8],
                        esc[:, khi * 8:(khi + 1) * 8],
                        rec[:, khi:khi + 1])

                # transpose attn -> [32(c=khi*8+n), 128(klo)] (padded to 32 for
                # the xbar), then gather to DRAM n-major: d[n*256+khi*128+klo]
                attn_t = work.tile([128, 128], _DT.bfloat16, tag="attn_t")
                nc.sync.dma_start_transpose(attn_t[:], attn_pad[:])
                aflat_d = dram.tile([BJ], _DT.bfloat16, tag="aflat")
                for khi in range(2):
                    af_out = bass.AP(tensor=aflat_d.tensor,
                                     offset=aflat_d[:].offset + khi * 128,
                                     ap=[[256, 8], [1, 128]])
                    nc.sync.dma_start(af_out, attn_t[khi * 8:(khi + 1) * 8, :])
                # broadcast to all 128 partitions in one DMA (DRAM src, step-0)
                attn_b = work.tile([128, BJ], _DT.bfloat16, tag="attnb")
                ab_in = bass.AP(tensor=aflat_d.tensor, offset=aflat_d[:].offset,
                                ap=[[0, 128], [1, BJ]])
                nc.sync.dma_start(attn_b[:], ab_in)

                # weighted sum over n: pair tree on val * attn
                wp = work.tile([128, BJ], _DT.bfloat16, tag="wp")
                nc.vector.tensor_mul(wp[:], valT[:], attn_b[:])
                wt1 = work.tile([128, 1024], _DT.bfloat16, tag="wt1")
                nc.vector.tensor_add(wt1[:], wp[:, 0:1024], wp[:, 1024:2048])
                wt2 = work.tile([128, 512], _DT.bfloat16, tag="wt2")
                nc.vector.tensor_add(wt2[:], wt1[:, 0:512], wt1[:, 512:1024])
                nc.vector.tensor_add(aaf_s[:, bi * 256:(bi + 1) * 256],
                                     wt2[:, 0:256], wt2[:, 256:512])

              # ---- super-block (4 blocks = 1024 agents, 128 groups) ----
              nc.gpsimd.dma_start(aa_d[:, sb * 1024:(sb + 1) * 1024], aaf_s[:])
              nc.vector.tensor_copy(aab_s[:], aaf_s[:])

              # agent MLP on [128, 1024]
              h1a = work.tile([128, 1024], _DT.bfloat16, tag="h1a")
              psa = l2_ps.tile([128, 1024], _DT.float32, tag="l2")
              nc.tensor.matmul(psa[:, 0:512], w1a[:], aab_s[:, 0:512])
              nc.tensor.matmul(psa[:, 512:1024], w1a[:], aab_s[:, 512:1024])
              nc.scalar.activation(h1a[:], psa[:], _AF.Tanh, bias=biases[:, 4:5])
              avT = work.tile([128, 1024], _DT.bfloat16, tag="avT")
              psa2 = l2_ps.tile([128, 1024], _DT.float32, tag="l2")
              nc.tensor.matmul(psa2[:, 0:512], w2a[:], h1a[:, 0:512])
              nc.tensor.matmul(psa2[:, 512:1024], w2a[:], h1a[:, 512:1024])
              nc.scalar.activation(avT[:], psa2[:], _AF.Tanh, bias=biases[:, 5:6])

              # level-2 attention (cols are k = 8g + a_agent, 128 groups)
              aab_ga = aab_s[:].rearrange("p (g a) -> p g a", g=128)
              q21 = work.tile([128, 128, 4], _DT.bfloat16, tag="q21")
              nc.vector.tensor_add(q21[:], aab_ga[:, :, 0:4], aab_ga[:, :, 4:8])
              q22 = work.tile([128, 128, 2], _DT.bfloat16, tag="q22")
              nc.vector.tensor_add(q22[:], q21[:, :, 0:2], q21[:, :, 2:4])
              q2s = work.tile([128, 128], _DT.bfloat16, tag="q2s")
              q2s_v = q2s[:].rearrange("p (g o) -> p g o", o=1)
              nc.vector.tensor_add(q2s_v, q22[:, :, 0:1], q22[:, :, 1:2])

              prod2 = work.tile([128, 1024], _DT.bfloat16, tag="prod2")
              prod2_ga = prod2[:].rearrange("p (g a) -> p g a", g=128)
              nc.vector.tensor_mul(prod2_ga, aab_ga, _bc(q2s[:], 8, 2))

              e2s = work.tile([8, 1024], _DT.float32, tag="e2s")
              ps2 = l2_ps.tile([8, 1024], _DT.float32, tag="l2")
              nc.tensor.matmul(ps2[:, 0:512], ihd[:], prod2[:, 0:512])
              nc.tensor.matmul(ps2[:, 512:1024], ihd[:], prod2[:, 512:1024])
              nc.scalar.activation(e2s[:], ps2[:], _AF.Exp, scale=S2)
              den2 = work.tile([8, 128], _DT.float32, tag="den2")
              e2s_v = e2s[:].rearrange("p (g a) -> p g a", g=128)
              nc.vector.tensor_reduce(den2[:], e2s_v, axis=mybir.AxisListType.X,
                                      op=_OP.add)
              rec2 = work.tile([8, 128], _DT.float32, tag="rec2")
              nc.vector.reciprocal(rec2[:], den2[:])
              attn2 = work.tile([8, 1024], _DT.bfloat16, tag="attn2")
              attn2_v = attn2[:].rearrange("p (g a) -> p g a", g=128)
              nc.vector.tensor_mul(attn2_v, e2s_v, _bc(rec2[:], 8, 2))

              # broadcast attn2[hd, :] to partitions [16hd:16hd+16) on PE:
              # a2b[p, c] = sum_hd ihdT[hd, p] * attn2[hd, c] = attn2[p//16, c]
              wp2 = work.tile([128, 1024], _DT.bfloat16, tag="wp2")
              a2b = l2_ps.tile([128, 1024], _DT.float32, tag="l2")
              nc.tensor.matmul(a2b[:, 0:512], ihdT[:], attn2[:, 0:512])
              nc.tensor.matmul(a2b[:, 512:1024], ihdT[:], attn2[:, 512:1024])
              nc.vector.tensor_mul(wp2[:], avT[:], a2b[:])
              wp2_ga = wp2[:].rearrange("p (g a) -> p g a", g=128)
              o21 = work.tile([128, 128, 4], _DT.bfloat16, tag="o21")
              nc.vector.tensor_add(o21[:], wp2_ga[:, :, 0:4], wp2_ga[:, :, 4:8])
              o22 = work.tile([128, 128, 2], _DT.bfloat16, tag="o22")
              nc.vector.tensor_add(o22[:], o21[:, :, 0:2], o21[:, :, 2:4])
              o2f = work.tile([128, 128], _DT.float32, tag="o2f")
              o2f_v = o2f[:].rearrange("p (g o) -> p g o", o=1)
              nc.vector.tensor_add(o2f_v, o22[:, :, 0:1], o22[:, :, 1:2])
              nc.gpsimd.dma_start(out2_d[:, sb * 128:(sb + 1) * 128], o2f[:])

    nc.compile()
    return nc


def pack_core_inputs(obs, weights, core, n_blocks=N_BLOCKS_FULL):
    """Build the per-core input dict. obs: [65536, 114] fp32."""
    NK = n_blocks * BK
    J = NK * NUM_ADV
    self18 = obs[:, :SELF_OBS_DIM]
    p = np.arange(J)
    b = p // BJ
    jl = p % BJ
    n = jl // BK
    kib = jl % BK
    r = NUM_ADV * (b * BK + kib) + n          # local mlp row (== global self row)
    kl = b * BK + kib                          # local batch row
    nbr = obs[NB_PER_CORE * core: NB_PER_CORE * core + NK,
              SELF_OBS_DIM:SELF_OBS_DIM + NUM_ADV * NBR_OBS_DIM]
    nbr = nbr.reshape(NK, NUM_ADV, NBR_OBS_DIM)
    feat = np.empty((J, 30), np.float32)
    feat[:, :18] = self18[r]
    feat[:, 18:] = nbr[kl, n]
    X = feat.reshape(n_blocks, 4, 512, 30).transpose(1, 3, 0, 2)  # [q, f, b, c]
    inp = np.zeros((4, 32, n_blocks, 512), np.float32)
    inp[:, :30] = X
    inp = inp.reshape(128, n_blocks * 512).astype(BF16)

    (eW1, eb1, eW2, eb2, vW1, vb1, vW2, vb2, aW1, ab1, aW2, ab2) = weights
    w1e = np.zeros((128, 128), np.float32)
    for q in range(4):
        w1e[32 * q:32 * q + 30] = eW1
    bias = np.stack([eb1, eb2, vb1, vb2, ab1, ab2], axis=1).astype(np.float32)
    ihd = np.zeros((128, 8), np.float32)
    for hd in range(8):
        ihd[16 * hd:16 * (hd + 1), hd] = 1.0
    return {
        "inp": inp,
        "w1e": w1e.astype(BF16),
        "w2e": eW2.astype(BF16),
        "w1v": vW1.astype(BF16),
        "w2v": vW2.astype(BF16),
        "w1a": aW1.astype(BF16),
        "w2a": aW2.astype(BF16),
        "bias": bias,
        "ones": np.ones((128, 1), BF16),
        "ihd": ihd.astype(BF16),
        "ident": np.eye(128, dtype=np.float32).astype(BF16),
        "ihdT": ihd.T.copy().astype(BF16),
    }


_NC_CACHE = {}


def _get_nc(n_blocks=N_BLOCKS_FULL):
    if n_blocks not in _NC_CACHE:
        _NC_CACHE[n_blocks] = build_bass(n_blocks)
    return _NC_CACHE[n_blocks]


def run_cores(obs, weights, n_blocks=N_BLOCKS_FULL, trace=False, **kw):
    nc = _get_nc(n_blocks)
    in_maps = [pack_core_inputs(obs, weights, d, n_blocks) for d in range(NCORES)]
    res = run_bass_kernel_spmd(nc, in_maps, core_ids=list(range(NCORES)),
                               trace=trace, **kw)
    return res


def kernel(obs, eW1, eb1, eW2, eb2, vW1, vb1, vW2, vb2, aW1, ab1, aW2, ab2,
           adv_obs_size=None, all_adv_obs_size=None, batch_size=None,
           num_groups=None, _trace=False, _res_out=None):
    obs = np.asarray(obs, dtype=np.float32)
    weights = tuple(np.asarray(w, dtype=np.float32)
                    for w in (eW1, eb1, eW2, eb2, vW1, vb1, vW2, vb2,
                              aW1, ab1, aW2, ab2))
    res = run_cores(obs, weights, trace=_trace)
    if _res_out is not None:
        _res_out.append(res)
    aa = np.empty((BATCH, HID), np.float32)
    out2 = np.empty((BATCH // NUM_AGENTS, HID), np.float32)
    for d in range(NCORES):
        aa[NB_PER_CORE * d:NB_PER_CORE * (d + 1)] = res.results[d]["aa"].T
        gd = NB_PER_CORE // NUM_AGENTS
        out2[gd * d:gd * (d + 1)] = res.results[d]["out2"].T
    multi_head = np.tile(out2, (NUM_AGENTS, 1))
    return multi_head, aa


# revision 28
# speedup vs baseline: 1.0914x; 1.0914x over previous
"""Trainium2 Bass kernel for nn_Critic_QuadAdv_MultiheadAttention.

Self-contained: accepts FULL inputs (as produced by setup_inputs()), shards
across 8 NeuronCores (pure data parallel over batch), runs one fused Bass
kernel per core, gathers and returns the full output tuple
(multi_head_attention [65536,128], agent_attention [65536,128]).
"""

import sys

sys.path.insert(0, "/opt/trn_rl_repo")

import numpy as np
import ml_dtypes

import concourse.bass as bass
import concourse.tile as tile
from concourse import bacc, mybir
from concourse.bass_utils import run_bass_kernel_spmd

BF16 = ml_dtypes.bfloat16

# Model constants (hardcoded per spec)
NUM_HEADS = 8
ATTN_SIZE = 16
NUM_AGENTS = 8
NBR_OBS_DIM = 12
SELF_OBS_DIM = 18
NUM_ADV = 8
HID = 128
BATCH = 65536
NCORES = 8
NB_PER_CORE = BATCH // NCORES          # 8192 batch rows per core
BK = 256                               # batch rows per block
BJ = BK * NUM_ADV                      # 2048 mlp rows per block
N_BLOCKS_FULL = NB_PER_CORE // BK      # 32

_DT = mybir.dt
_AF = mybir.ActivationFunctionType
_OP = mybir.AluOpType


def _bc(ap: bass.AP, n: int, axis: int) -> bass.AP:
    """Insert a broadcast (step 0, count n) free dim at position `axis` of ap.ap."""
    new = list(ap.ap)
    new.insert(axis, [0, n])
    return bass.AP(tensor=ap.tensor, offset=ap.offset, ap=new)


def build_bass(n_blocks: int = N_BLOCKS_FULL):
    nc = bacc.Bacc(None, target_bir_lowering=False)
    NK = n_blocks * BK          # batch rows this core
    NG = NK // NUM_AGENTS       # groups this core
    NCOL = n_blocks * 512       # packed input cols

    inp_d = nc.dram_tensor("inp", [128, NCOL], _DT.bfloat16, kind="ExternalInput")
    w1e_d = nc.dram_tensor("w1e", [128, 128], _DT.bfloat16, kind="ExternalInput")
    w2e_d = nc.dram_tensor("w2e", [128, 128], _DT.bfloat16, kind="ExternalInput")
    w1v_d = nc.dram_tensor("w1v", [128, 128], _DT.bfloat16, kind="ExternalInput")
    w2v_d = nc.dram_tensor("w2v", [128, 128], _DT.bfloat16, kind="ExternalInput")
    w1a_d = nc.dram_tensor("w1a", [128, 128], _DT.bfloat16, kind="ExternalInput")
    w2a_d = nc.dram_tensor("w2a", [128, 128], _DT.bfloat16, kind="ExternalInput")
    bias_d = nc.dram_tensor("bias", [128, 6], _DT.float32, kind="ExternalInput")
    ones_d = nc.dram_tensor("ones", [128, 1], _DT.bfloat16, kind="ExternalInput")
    ihd_d = nc.dram_tensor("ihd", [128, 8], _DT.bfloat16, kind="ExternalInput")
    ident_d = nc.dram_tensor("ident", [128, 128], _DT.bfloat16, kind="ExternalInput")
    ihdT_d = nc.dram_tensor("ihdT", [8, 128], _DT.bfloat16, kind="ExternalInput")
    aa_d = nc.dram_tensor("aa", [128, NK], _DT.float32, kind="ExternalOutput")
    out2_d = nc.dram_tensor("out2", [128, NG], _DT.float32, kind="ExternalOutput")

    S1 = 1.0 / (NUM_ADV * float(np.sqrt(HID)))      # level-1: mean/8 and /sqrt(128)
    S2 = 1.0 / (NUM_AGENTS * float(np.sqrt(ATTN_SIZE)))  # level-2: mean/8 and /4

    with tile.TileContext(nc) as tc:
        with (
            tc.tile_pool(name="const", bufs=1) as constp,
            tc.tile_pool(name="io", bufs=4) as iop,
            tc.tile_pool(name="work", bufs=3) as work,
            tc.tile_pool(name="mlp_ps", bufs=2, space="PSUM") as mlp_ps,
            tc.tile_pool(name="sc_ps", bufs=2, space="PSUM") as sc_ps,
            tc.tile_pool(name="l2_ps", bufs=1, space="PSUM") as l2_ps,
            tc.tile_pool(name="dram", bufs=2, space="DRAM") as dram,
        ):
            w1e = constp.tile([128, 128], _DT.bfloat16)
            w2e = constp.tile([128, 128], _DT.bfloat16)
            w1v = constp.tile([128, 128], _DT.bfloat16)
            w2v = constp.tile([128, 128], _DT.bfloat16)
            w1a = constp.tile([128, 128], _DT.bfloat16)
            w2a = constp.tile([128, 128], _DT.bfloat16)
            biases = constp.tile([128, 6], _DT.float32)
            ones = constp.tile([128, 1], _DT.bfloat16)
            ihd = constp.tile([128, 8], _DT.bfloat16)
            ident = constp.tile([128, 128], _DT.bfloat16)
            ihdT = constp.tile([8, 128], _DT.bfloat16)
            nc.sync.dma_start(w1e[:], w1e_d[:])
            nc.sync.dma_start(w2e[:], w2e_d[:])
            nc.sync.dma_start(w1v[:], w1v_d[:])
            nc.sync.dma_start(w2v[:], w2v_d[:])
            nc.sync.dma_start(w1a[:], w1a_d[:])
            nc.sync.dma_start(w2a[:], w2a_d[:])
            nc.sync.dma_start(biases[:], bias_d[:])
            nc.sync.dma_start(ones[:], ones_d[:])
            nc.sync.dma_start(ihd[:], ihd_d[:])
            nc.sync.dma_start(ident[:], ident_d[:])
            nc.sync.dma_start(ihdT[:], ihdT_d[:])

            def mlp_layer(dst, src_fn, w, bias_col, first=False, xin=None):
                """One [128->128] layer over BJ cols: matmul halves + tanh."""
                for half in range(2):
                    ps = mlp_ps.tile([128, 1024], _DT.float32, tag="mlp")
                    for qq in range(2):
                        if first:
                            q = half * 2 + qq
                            nc.tensor.matmul(
                                ps[:, qq * 512:(qq + 1) * 512],
                                w[32 * q:32 * (q + 1), :],
                                xin[32 * q:32 * (q + 1), :],
                                tile_position=(32 * q, 0),
                            )
                        else:
                            s = half * 1024 + qq * 512
                            nc.tensor.matmul(
                                ps[:, qq * 512:(qq + 1) * 512],
                                w[:],
                                src_fn[:, s:s + 512],
                            )
                    nc.scalar.activation(
                        dst[:, half * 1024:(half + 1) * 1024],
                        ps[:],
                        _AF.Tanh,
                        bias=biases[:, bias_col:bias_col + 1],
                    )

            assert n_blocks % 4 == 0
            for sb in range(n_blocks // 4):
              aaf_s = work.tile([128, 1024], _DT.float32, tag="aafs")
              aab_s = work.tile([128, 1024], _DT.bfloat16, tag="aabs")
              for bi in range(4):
                blk = sb * 4 + bi
                xin = iop.tile([128, 512], _DT.bfloat16, tag="xin")
                nc.gpsimd.dma_start(xin[:], inp_d[:, blk * 512:(blk + 1) * 512])

                h1e = work.tile([128, BJ], _DT.bfloat16, tag="h1e")
                embT = work.tile([128, BJ], _DT.bfloat16, tag="embT")
                h1v = work.tile([128, BJ], _DT.bfloat16, tag="h1v")
                valT = work.tile([128, BJ], _DT.bfloat16, tag="valT")

                mlp_layer(h1e, None, w1e, 0, first=True, xin=xin)
                mlp_layer(embT, h1e, w2e, 1)
                mlp_layer(h1v, embT, w1v, 2)
                mlp_layer(valT, h1v, w2v, 3)

                # ---- level-1 attention (cols are n-major: j = n*256 + k) ----
                # q_sum[h,k] = sum_n emb[h, n*256+k]  (pair tree, contiguous halves)
                qs1 = work.tile([128, 1024], _DT.bfloat16, tag="qs1")
                nc.vector.tensor_add(qs1[:], embT[:, 0:1024], embT[:, 1024:2048])
                qs2 = work.tile([128, 512], _DT.bfloat16, tag="qs2")
                nc.vector.tensor_add(qs2[:], qs1[:, 0:512], qs1[:, 512:1024])
                qsum = work.tile([128, 256], _DT.bfloat16, tag="qsum")
                nc.vector.tensor_add(qsum[:], qs2[:, 0:256], qs2[:, 256:512])

                # prod[h, (n,k)] = emb[h,(n,k)] * q_sum[h,k]
                prod = work.tile([128, BJ], _DT.bfloat16, tag="prod")
                emb_nk = embT[:].rearrange("p (n k) -> p n k", n=8)
                prod_nk = prod[:].rearrange("p (n k) -> p n k", n=8)
                nc.vector.tensor_mul(prod_nk, emb_nk, _bc(qsum[:], 8, 1))

                # scores via prod-as-weights: psum_sc[p, t] = sum_h prod[h, t*128+p]
                # col j = t*128+p  ->  n = t//2, khi = t%2, klo = p
                # col order: (khi, n): ssp[:, khi*8+n] = scores for k=khi*128+klo
                ssp = sc_ps.tile([128, 16], _DT.float32, tag="sc")
                for t in range(16):
                    n_, khi_ = t // 2, t % 2
                    c = khi_ * 8 + n_
                    nc.tensor.matmul(
                        ssp[:, c:c + 1],
                        prod[:, t * 128:(t + 1) * 128],
                        ones[:],
                    )
                # exp with fused scale -> esc[klo, (khi,n)]
                esc = work.tile([128, 16], _DT.float32, tag="esc")
                nc.scalar.activation(esc[:], ssp[:], _AF.Exp, scale=S1)
                # denom over n per khi
                den = work.tile([128, 2], _DT.float32, tag="den")
                esc_kn = esc[:].rearrange("p (khi n) -> p khi n", khi=2)
                nc.vector.tensor_reduce(den[:], esc_kn, axis=mybir.AxisListType.X,
                                        op=_OP.add)
                rec = work.tile([128, 2], _DT.float32, tag="rec")
                nc.vector.reciprocal(rec[:], den[:])
                # attn[klo, (khi,n)] = esc * rec[khi]
                attn = work.tile([128, 16], _DT.bfloat16, tag="attn")
                for khi in range(2):
                    nc.vector.tensor_scalar_mul(
                        attn[:, khi * 8:(khi + 1) * 8],
                        esc[:, khi * 8:(khi + 1) * 8],
                        rec[:, khi:khi + 1])

                # transpose attn on PE -> psum [16, 128(klo)], evac to sbuf,
                # then gather to DRAM n-major: d[n*256+khi*128+klo]
                att_ps = sc_ps.tile([16, 128], _DT.bfloat16, tag="sc")
                nc.tensor.transpose(att_ps[:], attn[:], ident[:])
                attn_t = work.tile([16, 128], _DT.bfloat16, tag="attn_t")
                nc.vector.tensor_copy(attn_t[:], att_ps[:])
                aflat_d = dram.tile([BJ], _DT.bfloat16, tag="aflat")
                for khi in range(2):
                    af_out = bass.AP(tensor=aflat_d.tensor,
                                     offset=aflat_d[:].offset + khi * 128,
                                     ap=[[256, 8], [1, 128]])
                    nc.sync.dma_start(af_out, attn_t[khi * 8:(khi + 1) * 8, :])
                # broadcast to all 128 partitions in one DMA (DRAM src, step-0)
                attn_b = work.tile([128, BJ], _DT.bfloat16, tag="attnb")
                ab_in = bass.AP(tensor=aflat_d.tensor, offset=aflat_d[:].offset,
                                ap=[[0, 128], [1, BJ]])
                nc.sync.dma_start(attn_b[:], ab_in)

                # weighted sum over n: pair tree on val * attn
                wp = work.tile([128, BJ], _DT.bfloat16, tag="wp")
                nc.vector.tensor_mul(wp[:], valT[:], attn_b[:])
                wt1 = work.tile([128, 1024], _DT.bfloat16, tag="wt1")
                nc.vector.tensor_add(wt1[:], wp[:, 0:1024], wp[:, 1024:2048])
                wt2 = work.tile([128, 512], _DT.bfloat16, tag="wt2")
                nc.vector.tensor_add(wt2[:], wt1[:, 0:512], wt1[:, 512:1024])
                nc.vector.tensor_add(aaf_s[:, bi * 256:(bi + 1) * 256],
                                     wt2[:, 0:256], wt2[:, 256:512])

              # ---- super-block (4 blocks = 1024 agents, 128 groups) ----
              nc.gpsimd.dma_start(aa_d[:, sb * 1024:(sb + 1) * 1024], aaf_s[:])
              nc.vector.tensor_copy(aab_s[:], aaf_s[:])

              # agent MLP on [128, 1024]
              h1a = work.tile([128, 1024], _DT.bfloat16, tag="h1a")
              psa = l2_ps.tile([128, 1024], _DT.float32, tag="l2")
              nc.tensor.matmul(psa[:, 0:512], w1a[:], aab_s[:, 0:512])
              nc.tensor.matmul(psa[:, 512:1024], w1a[:], aab_s[:, 512:1024])
              nc.scalar.activation(h1a[:], psa[:], _AF.Tanh, bias=biases[:, 4:5])
              avT = work.tile([128, 1024], _DT.bfloat16, tag="avT")
              psa2 = l2_ps.tile([128, 1024], _DT.float32, tag="l2")
              nc.tensor.matmul(psa2[:, 0:512], w2a[:], h1a[:, 0:512])
              nc.tensor.matmul(psa2[:, 512:1024], w2a[:], h1a[:, 512:1024])
              nc.scalar.activation(avT[:], psa2[:], _AF.Tanh, bias=biases[:, 5:6])

              # level-2 attention (cols are k = 8g + a_agent, 128 groups)
              aab_ga = aab_s[:].rearrange("p (g a) -> p g a", g=128)
              q21 = work.tile([128, 128, 4], _DT.bfloat16, tag="q21")
              nc.vector.tensor_add(q21[:], aab_ga[:, :, 0:4], aab_ga[:, :, 4:8])
              q22 = work.tile([128, 128, 2], _DT.bfloat16, tag="q22")
              nc.vector.tensor_add(q22[:], q21[:, :, 0:2], q21[:, :, 2:4])
              q2s = work.tile([128, 128], _DT.bfloat16, tag="q2s")
              q2s_v = q2s[:].rearrange("p (g o) -> p g o", o=1)
              nc.vector.tensor_add(q2s_v, q22[:, :, 0:1], q22[:, :, 1:2])

              prod2 = work.tile([128, 1024], _DT.bfloat16, tag="prod2")
              prod2_ga = prod2[:].rearrange("p (g a) -> p g a", g=128)
              nc.vector.tensor_mul(prod2_ga, aab_ga, _bc(q2s[:], 8, 2))

              e2s = work.tile([8, 1024], _DT.float32, tag="e2s")
              ps2 = l2_ps.tile([8, 1024], _DT.float32, tag="l2")
              nc.tensor.matmul(ps2[:, 0:512], ihd[:], prod2[:, 0:512])
              nc.tensor.matmul(ps2[:, 512:1024], ihd[:], prod2[:, 512:1024])
              nc.scalar.activation(e2s[:], ps2[:], _AF.Exp, scale=S2)
              den2 = work.tile([8, 128], _DT.float32, tag="den2")
              e2s_v = e2s[:].rearrange("p (g a) -> p g a", g=128)
              nc.vector.tensor_reduce(den2[:], e2s_v, axis=mybir.AxisListType.X,
                                      op=_OP.add)
              rec2 = work.tile([8, 128], _DT.float32, tag="rec2")
              nc.vector.reciprocal(rec2[:], den2[:])
              attn2 = work.tile([8, 1024], _DT.bfloat16, tag="attn2")
              attn2_v = attn2[:].rearrange("p (g a) -> p g a", g=128)
              nc.vector.tensor_mul(attn2_v, e2s_v, _bc(rec2[:], 8, 2))

              # broadcast attn2[hd, :] to partitions [16hd:16hd+16) on PE:
              # a2b[p, c] = sum_hd ihdT[hd, p] * attn2[hd, c] = attn2[p//16, c]
              wp2 = work.tile([128, 1024], _DT.bfloat16, tag="wp2")
              a2b = l2_ps.tile([128, 1024], _DT.float32, tag="l2")
              nc.tensor.matmul(a2b[:, 0:512], ihdT[:], attn2[:, 0:512])
              nc.tensor.matmul(a2b[:, 512:1024], ihdT[:], attn2[:, 512:1024])
              nc.vector.tensor_mul(wp2[:], avT[:], a2b[:])
              wp2_ga = wp2[:].rearrange("p (g a) -> p g a", g=128)
              o21 = work.tile([128, 128, 4], _DT.bfloat16, tag="o21")
              nc.vector.tensor_add(o21[:], wp2_ga[:, :, 0:4], wp2_ga[:, :, 4:8])
              o22 = work.tile([128, 128, 2], _DT.bfloat16, tag="o22")
              nc.vector.tensor_add(o22[:], o21[:, :, 0:2], o21[:, :, 2:4])
              o2f = work.tile([128, 128], _DT.float32, tag="o2f")
              o2f_v = o2f[:].rearrange("p (g o) -> p g o", o=1)
              nc.vector.tensor_add(o2f_v, o22[:, :, 0:1], o22[:, :, 1:2])
              nc.gpsimd.dma_start(out2_d[:, sb * 128:(sb + 1) * 128], o2f[:])

    nc.compile()
    return nc


def pack_core_inputs(obs, weights, core, n_blocks=N_BLOCKS_FULL):
    """Build the per-core input dict. obs: [65536, 114] fp32."""
    NK = n_blocks * BK
    J = NK * NUM_ADV
    self18 = obs[:, :SELF_OBS_DIM]
    p = np.arange(J)
    b = p // BJ
    jl = p % BJ
    n = jl // BK
    kib = jl % BK
    r = NUM_ADV * (b * BK + kib) + n          # local mlp row (== global self row)
    kl = b * BK + kib                          # local batch row
    nbr = obs[NB_PER_CORE * core: NB_PER_CORE * core + NK,
              SELF_OBS_DIM:SELF_OBS_DIM + NUM_ADV * NBR_OBS_DIM]
    nbr = nbr.reshape(NK, NUM_ADV, NBR_OBS_DIM)
    feat = np.empty((J, 30), np.float32)
    feat[:, :18] = self18[r]
    feat[:, 18:] = nbr[kl, n]
    X = feat.reshape(n_blocks, 4, 512, 30).transpose(1, 3, 0, 2)  # [q, f, b, c]
    inp = np.zeros((4, 32, n_blocks, 512), np.float32)
    inp[:, :30] = X
    inp = inp.reshape(128, n_blocks * 512).astype(BF16)

    (eW1, eb1, eW2, eb2, vW1, vb1, vW2, vb2, aW1, ab1, aW2, ab2) = weights
    w1e = np.zeros((128, 128), np.float32)
    for q in range(4):
        w1e[32 * q:32 * q + 30] = eW1
    bias = np.stack([eb1, eb2, vb1, vb2, ab1, ab2], axis=1).astype(np.float32)
    ihd = np.zeros((128, 8), np.float32)
    for hd in range(8):
        ihd[16 * hd:16 * (hd + 1), hd] = 1.0
    return {
        "inp": inp,
        "w1e": w1e.astype(BF16),
        "w2e": eW2.astype(BF16),
        "w1v": vW1.astype(BF16),
        "w2v": vW2.astype(BF16),
        "w1a": aW1.astype(BF16),
        "w2a": aW2.astype(BF16),
        "bias": bias,
        "ones": np.ones((128, 1), BF16),
        "ihd": ihd.astype(BF16),
        "ident": np.eye(128, dtype=np.float32).astype(BF16),
        "ihdT": ihd.T.copy().astype(BF16),
    }


_NC_CACHE = {}


def _get_nc(n_blocks=N_BLOCKS_FULL):
    if n_blocks not in _NC_CACHE:
        _NC_CACHE[n_blocks] = build_bass(n_blocks)
    return _NC_CACHE[n_blocks]


def run_cores(obs, weights, n_blocks=N_BLOCKS_FULL, trace=False, **kw):
    nc = _get_nc(n_blocks)
    in_maps = [pack_core_inputs(obs, weights, d, n_blocks) for d in range(NCORES)]
    res = run_bass_kernel_spmd(nc, in_maps, core_ids=list(range(NCORES)),
                               trace=trace, **kw)
    return res


def kernel(obs, eW1, eb1, eW2, eb2, vW1, vb1, vW2, vb2, aW1, ab1, aW2, ab2,
           adv_obs_size=None, all_adv_obs_size=None, batch_size=None,
           num_groups=None, _trace=False, _res_out=None):
    obs = np.asarray(obs, dtype=np.float32)
    weights = tuple(np.asarray(w, dtype=np.float32)
                    for w in (eW1, eb1, eW2, eb2, vW1, vb1, vW2, vb2,
                              aW1, ab1, aW2, ab2))
    res = run_cores(obs, weights, trace=_trace)
    if _res_out is not None:
        _res_out.append(res)
    aa = np.empty((BATCH, HID), np.float32)
    out2 = np.empty((BATCH // NUM_AGENTS, HID), np.float32)
    for d in range(NCORES):
        aa[NB_PER_CORE * d:NB_PER_CORE * (d + 1)] = res.results[d]["aa"].T
        gd = NB_PER_CORE // NUM_AGENTS
        out2[gd * d:gd * (d + 1)] = res.results[d]["out2"].T
    multi_head = np.tile(out2, (NUM_AGENTS, 1))
    return multi_head, aa


# revision 30
# speedup vs baseline: 1.0999x; 1.0077x over previous
"""Trainium2 Bass kernel for nn_Critic_QuadAdv_MultiheadAttention.

Self-contained: accepts FULL inputs (as produced by setup_inputs()), shards
across 8 NeuronCores (pure data parallel over batch), runs one fused Bass
kernel per core, gathers and returns the full output tuple
(multi_head_attention [65536,128], agent_attention [65536,128]).
"""

import sys

sys.path.insert(0, "/opt/trn_rl_repo")

import numpy as np
import ml_dtypes

import concourse.bass as bass
import concourse.tile as tile
from concourse import bacc, mybir
from concourse.bass_utils import run_bass_kernel_spmd

BF16 = ml_dtypes.bfloat16

# Model constants (hardcoded per spec)
NUM_HEADS = 8
ATTN_SIZE = 16
NUM_AGENTS = 8
NBR_OBS_DIM = 12
SELF_OBS_DIM = 18
NUM_ADV = 8
HID = 128
BATCH = 65536
NCORES = 8
NB_PER_CORE = BATCH // NCORES          # 8192 batch rows per core
BK = 256                               # batch rows per block
BJ = BK * NUM_ADV                      # 2048 mlp rows per block
N_BLOCKS_FULL = NB_PER_CORE // BK      # 32

_DT = mybir.dt
_AF = mybir.ActivationFunctionType
_OP = mybir.AluOpType


def _bc(ap: bass.AP, n: int, axis: int) -> bass.AP:
    """Insert a broadcast (step 0, count n) free dim at position `axis` of ap.ap."""
    new = list(ap.ap)
    new.insert(axis, [0, n])
    return bass.AP(tensor=ap.tensor, offset=ap.offset, ap=new)


def build_bass(n_blocks: int = N_BLOCKS_FULL):
    nc = bacc.Bacc(None, target_bir_lowering=False)
    NK = n_blocks * BK          # batch rows this core
    NG = NK // NUM_AGENTS       # groups this core
    NCOL = n_blocks * 512       # packed input cols

    inp_d = nc.dram_tensor("inp", [128, NCOL], _DT.bfloat16, kind="ExternalInput")
    w1e_d = nc.dram_tensor("w1e", [128, 128], _DT.bfloat16, kind="ExternalInput")
    w2e_d = nc.dram_tensor("w2e", [128, 128], _DT.bfloat16, kind="ExternalInput")
    w1v_d = nc.dram_tensor("w1v", [128, 128], _DT.bfloat16, kind="ExternalInput")
    w2v_d = nc.dram_tensor("w2v", [128, 128], _DT.bfloat16, kind="ExternalInput")
    w1a_d = nc.dram_tensor("w1a", [128, 128], _DT.bfloat16, kind="ExternalInput")
    w2a_d = nc.dram_tensor("w2a", [128, 128], _DT.bfloat16, kind="ExternalInput")
    bias_d = nc.dram_tensor("bias", [128, 6], _DT.float32, kind="ExternalInput")
    ones_d = nc.dram_tensor("ones", [128, 1], _DT.bfloat16, kind="ExternalInput")
    ihd_d = nc.dram_tensor("ihd", [128, 8], _DT.bfloat16, kind="ExternalInput")
    ident_d = nc.dram_tensor("ident", [128, 128], _DT.bfloat16, kind="ExternalInput")
    ihdT_d = nc.dram_tensor("ihdT", [8, 128], _DT.bfloat16, kind="ExternalInput")
    aa_d = nc.dram_tensor("aa", [128, NK], _DT.float32, kind="ExternalOutput")
    out2_d = nc.dram_tensor("out2", [128, NG], _DT.float32, kind="ExternalOutput")

    S1 = 1.0 / (NUM_ADV * float(np.sqrt(HID)))      # level-1: mean/8 and /sqrt(128)
    S2 = 1.0 / (NUM_AGENTS * float(np.sqrt(ATTN_SIZE)))  # level-2: mean/8 and /4

    with tile.TileContext(nc) as tc:
        with (
            tc.tile_pool(name="const", bufs=1) as constp,
            tc.tile_pool(name="io", bufs=4) as iop,
            tc.tile_pool(name="work", bufs=3) as work,
            tc.tile_pool(name="mlp_ps", bufs=2, space="PSUM") as mlp_ps,
            tc.tile_pool(name="sc_ps", bufs=2, space="PSUM") as sc_ps,
            tc.tile_pool(name="l2_ps", bufs=1, space="PSUM") as l2_ps,
            tc.tile_pool(name="dram", bufs=2, space="DRAM") as dram,
        ):
            w1e = constp.tile([128, 128], _DT.bfloat16)
            w2e = constp.tile([128, 128], _DT.bfloat16)
            w1v = constp.tile([128, 128], _DT.bfloat16)
            w2v = constp.tile([128, 128], _DT.bfloat16)
            w1a = constp.tile([128, 128], _DT.bfloat16)
            w2a = constp.tile([128, 128], _DT.bfloat16)
            biases = constp.tile([128, 6], _DT.float32)
            ones = constp.tile([128, 1], _DT.bfloat16)
            ihd = constp.tile([128, 8], _DT.bfloat16)
            ident = constp.tile([128, 128], _DT.bfloat16)
            ihdT = constp.tile([8, 128], _DT.bfloat16)
            nc.sync.dma_start(w1e[:], w1e_d[:])
            nc.sync.dma_start(w2e[:], w2e_d[:])
            nc.sync.dma_start(w1v[:], w1v_d[:])
            nc.sync.dma_start(w2v[:], w2v_d[:])
            nc.sync.dma_start(w1a[:], w1a_d[:])
            nc.sync.dma_start(w2a[:], w2a_d[:])
            nc.sync.dma_start(biases[:], bias_d[:])
            nc.sync.dma_start(ones[:], ones_d[:])
            nc.sync.dma_start(ihd[:], ihd_d[:])
            nc.sync.dma_start(ident[:], ident_d[:])
            nc.sync.dma_start(ihdT[:], ihdT_d[:])

            def mlp_layer(dst, src_fn, w, bias_col, first=False, xin=None):
                """One [128->128] layer over BJ cols: matmul halves + tanh."""
                for half in range(2):
                    ps = mlp_ps.tile([128, 1024], _DT.float32, tag="mlp")
                    for qq in range(2):
                        if first:
                            q = half * 2 + qq
                            nc.tensor.matmul(
                                ps[:, qq * 512:(qq + 1) * 512],
                                w[32 * q:32 * (q + 1), :],
                                xin[32 * q:32 * (q + 1), :],
                                tile_position=(32 * q, 0),
                            )
                        else:
                            s = half * 1024 + qq * 512
                            nc.tensor.matmul(
                                ps[:, qq * 512:(qq + 1) * 512],
                                w[:],
                                src_fn[:, s:s + 512],
                            )
                    nc.scalar.activation(
                        dst[:, half * 1024:(half + 1) * 1024],
                        ps[:],
                        _AF.Tanh,
                        bias=biases[:, bias_col:bias_col + 1],
                    )

            assert n_blocks % 4 == 0
            for sb in range(n_blocks // 4):
              aaf_s = work.tile([128, 1024], _DT.float32, tag="aafs")
              aab_s = work.tile([128, 1024], _DT.bfloat16, tag="aabs")
              for bi in range(4):
                blk = sb * 4 + bi
                xin = iop.tile([128, 512], _DT.bfloat16, tag="xin")
                nc.gpsimd.dma_start(xin[:], inp_d[:, blk * 512:(blk + 1) * 512])

                h1e = work.tile([128, BJ], _DT.bfloat16, tag="h1e")
                embT = work.tile([128, BJ], _DT.bfloat16, tag="embT")
                h1v = work.tile([128, BJ], _DT.bfloat16, tag="h1v")
                valT = work.tile([128, BJ], _DT.bfloat16, tag="valT")

                mlp_layer(h1e, None, w1e, 0, first=True, xin=xin)
                mlp_layer(embT, h1e, w2e, 1)
                mlp_layer(h1v, embT, w1v, 2)
                mlp_layer(valT, h1v, w2v, 3)

                # ---- level-1 attention (cols are n-major: j = n*256 + k) ----
                # q_sum[h,k] = sum_n emb[h, n*256+k]  (pair tree, contiguous halves)
                qs1 = work.tile([128, 1024], _DT.bfloat16, tag="qs1")
                nc.vector.tensor_add(qs1[:], embT[:, 0:1024], embT[:, 1024:2048])
                qs2 = work.tile([128, 512], _DT.bfloat16, tag="qs2")
                nc.vector.tensor_add(qs2[:], qs1[:, 0:512], qs1[:, 512:1024])
                qsum = work.tile([128, 256], _DT.bfloat16, tag="qsum")
                nc.vector.tensor_add(qsum[:], qs2[:, 0:256], qs2[:, 256:512])

                # prod[h, (n,k)] = emb[h,(n,k)] * q_sum[h,k]
                prod = work.tile([128, BJ], _DT.bfloat16, tag="prod")
                emb_nk = embT[:].rearrange("p (n k) -> p n k", n=8)
                prod_nk = prod[:].rearrange("p (n k) -> p n k", n=8)
                nc.vector.tensor_mul(prod_nk, emb_nk, _bc(qsum[:], 8, 1))

                # scores via prod-as-weights: psum_sc[p, t] = sum_h prod[h, t*128+p]
                # col j = t*128+p  ->  n = t//2, khi = t%2, klo = p
                # col order: (khi, n): ssp[:, khi*8+n] = scores for k=khi*128+klo
                ssp = sc_ps.tile([128, 16], _DT.float32, tag="sc")
                for t in range(16):
                    n_, khi_ = t // 2, t % 2
                    c = khi_ * 8 + n_
                    nc.tensor.matmul(
                        ssp[:, c:c + 1],
                        prod[:, t * 128:(t + 1) * 128],
                        ones[:],
                    )
                # exp with fused scale -> esc[klo, (khi,n)]
                esc = work.tile([128, 16], _DT.float32, tag="esc")
                nc.scalar.activation(esc[:], ssp[:], _AF.Exp, scale=S1)
                # denom over n per khi
                den = work.tile([128, 2], _DT.float32, tag="den")
                esc_kn = esc[:].rearrange("p (khi n) -> p khi n", khi=2)
                nc.vector.tensor_reduce(den[:], esc_kn, axis=mybir.AxisListType.X,
                                        op=_OP.add)
                rec = work.tile([128, 2], _DT.float32, tag="rec")
                nc.vector.reciprocal(rec[:], den[:])
                # attn[klo, (khi,n)] = esc * rec[khi]
                attn = work.tile([128, 16], _DT.bfloat16, tag="attn")
                for khi in range(2):
                    nc.vector.tensor_scalar_mul(
                        attn[:, khi * 8:(khi + 1) * 8],
                        esc[:, khi * 8:(khi + 1) * 8],
                        rec[:, khi:khi + 1])

                # transpose attn on PE -> psum [16, 128(klo)], evac to sbuf,
                # then gather to DRAM n-major: d[n*256+khi*128+klo]
                att_ps = sc_ps.tile([16, 128], _DT.bfloat16, tag="sc")
                nc.tensor.transpose(att_ps[:], attn[:], ident[:])
                attn_t = work.tile([16, 128], _DT.bfloat16, tag="attn_t")
                nc.vector.tensor_copy(attn_t[:], att_ps[:])
                aflat_d = dram.tile([BJ], _DT.bfloat16, tag="aflat")
                for khi in range(2):
                    af_out = bass.AP(tensor=aflat_d.tensor,
                                     offset=aflat_d[:].offset + khi * 128,
                                     ap=[[256, 8], [1, 128]])
                    nc.sync.dma_start(af_out, attn_t[khi * 8:(khi + 1) * 8, :])
                # broadcast to all 128 partitions in one DMA (DRAM src, step-0)
                attn_b = work.tile([128, BJ], _DT.bfloat16, tag="attnb")
                ab_in = bass.AP(tensor=aflat_d.tensor, offset=aflat_d[:].offset,
                                ap=[[0, 128], [1, BJ]])
                nc.sync.dma_start(attn_b[:], ab_in)

                # weighted sum over n: pair tree on val * attn
                wp = work.tile([128, BJ], _DT.bfloat16, tag="wp")
                nc.vector.tensor_mul(wp[:], valT[:], attn_b[:])
                wt1 = work.tile([128, 1024], _DT.bfloat16, tag="wt1")
                nc.vector.tensor_add(wt1[:], wp[:, 0:1024], wp[:, 1024:2048])
                wt2 = work.tile([128, 512], _DT.bfloat16, tag="wt2")
                nc.vector.tensor_add(wt2[:], wt1[:, 0:512], wt1[:, 512:1024])
                nc.vector.tensor_add(aaf_s[:, bi * 256:(bi + 1) * 256],
                                     wt2[:, 0:256], wt2[:, 256:512])

              # ---- super-block (4 blocks = 1024 agents, 128 groups) ----
              nc.gpsimd.dma_start(aa_d[:, sb * 1024:(sb + 1) * 1024], aaf_s[:])
              nc.vector.tensor_copy(aab_s[:], aaf_s[:])

              # agent MLP on [128, 1024]
              h1a = work.tile([128, 1024], _DT.bfloat16, tag="h1a")
              psa = l2_ps.tile([128, 1024], _DT.float32, tag="l2")
              nc.tensor.matmul(psa[:, 0:512], w1a[:], aab_s[:, 0:512])
              nc.tensor.matmul(psa[:, 512:1024], w1a[:], aab_s[:, 512:1024])
              nc.scalar.activation(h1a[:], psa[:], _AF.Tanh, bias=biases[:, 4:5])
              avT = work.tile([128, 1024], _DT.bfloat16, tag="avT")
              psa2 = l2_ps.tile([128, 1024], _DT.float32, tag="l2")
              nc.tensor.matmul(psa2[:, 0:512], w2a[:], h1a[:, 0:512])
              nc.tensor.matmul(psa2[:, 512:1024], w2a[:], h1a[:, 512:1024])
              nc.scalar.activation(avT[:], psa2[:], _AF.Tanh, bias=biases[:, 5:6])

              # level-2 attention (cols are k = 8g + a_agent, 128 groups)
              aab_ga = aab_s[:].rearrange("p (g a) -> p g a", g=128)
              q21 = work.tile([128, 128, 4], _DT.bfloat16, tag="q21")
              nc.vector.tensor_add(q21[:], aab_ga[:, :, 0:4], aab_ga[:, :, 4:8])
              q22 = work.tile([128, 128, 2], _DT.bfloat16, tag="q22")
              nc.vector.tensor_add(q22[:], q21[:, :, 0:2], q21[:, :, 2:4])
              q2s = work.tile([128, 128], _DT.bfloat16, tag="q2s")
              q2s_v = q2s[:].rearrange("p (g o) -> p g o", o=1)
              nc.vector.tensor_add(q2s_v, q22[:, :, 0:1], q22[:, :, 1:2])

              prod2 = work.tile([128, 1024], _DT.bfloat16, tag="prod2")
              prod2_ga = prod2[:].rearrange("p (g a) -> p g a", g=128)
              nc.vector.tensor_mul(prod2_ga, aab_ga, _bc(q2s[:], 8, 2))

              e2s = work.tile([8, 1024], _DT.float32, tag="e2s")
              ps2 = l2_ps.tile([8, 1024], _DT.float32, tag="l2")
              nc.tensor.matmul(ps2[:, 0:512], ihd[:], prod2[:, 0:512])
              nc.tensor.matmul(ps2[:, 512:1024], ihd[:], prod2[:, 512:1024])
              nc.scalar.activation(e2s[:], ps2[:], _AF.Exp, scale=S2)
              den2 = work.tile([8, 128], _DT.float32, tag="den2")
              e2s_v = e2s[:].rearrange("p (g a) -> p g a", g=128)
              nc.vector.tensor_reduce(den2[:], e2s_v, axis=mybir.AxisListType.X,
                                      op=_OP.add)
              rec2 = work.tile([8, 128], _DT.float32, tag="rec2")
              nc.vector.reciprocal(rec2[:], den2[:])
              attn2 = work.tile([8, 1024], _DT.bfloat16, tag="attn2")
              attn2_v = attn2[:].rearrange("p (g a) -> p g a", g=128)
              nc.vector.tensor_mul(attn2_v, e2s_v, _bc(rec2[:], 8, 2))

              # broadcast attn2[hd, :] to partitions [16hd:16hd+16) on PE:
              # a2b[p, c] = sum_hd ihdT[hd, p] * attn2[hd, c] = attn2[p//16, c]
              wp2 = work.tile([128, 1024], _DT.bfloat16, tag="wp2")
              a2b = l2_ps.tile([128, 1024], _DT.float32, tag="l2")
              nc.tensor.matmul(a2b[:, 0:512], ihdT[:], attn2[:, 0:512])
              nc.tensor.matmul(a2b[:, 512:1024], ihdT[:], attn2[:, 512:1024])
              nc.vector.tensor_mul(wp2[:], avT[:], a2b[:])
              wp2_ga = wp2[:].rearrange("p (g a) -> p g a", g=128)
              o21 = work.tile([128, 128, 4], _DT.bfloat16, tag="o21")
              nc.vector.tensor_add(o21[:], wp2_ga[:, :, 0:4], wp2_ga[:, :, 4:8])
              o22 = work.tile([128, 128, 2], _DT.bfloat16, tag="o22")
              nc.vector.tensor_add(o22[:], o21[:, :, 0:2], o21[:, :, 2:4])
              o2f = work.tile([128, 128], _DT.float32, tag="o2f")
              o2f_v = o2f[:].rearrange("p (g o) -> p g o", o=1)
              nc.vector.tensor_add(o2f_v, o22[:, :, 0:1], o22[:, :, 1:2])
              nc.gpsimd.dma_start(out2_d[:, sb * 128:(sb + 1) * 128], o2f[:])

    nc.compile()
    return nc


def pack_core_inputs(obs, weights, core, n_blocks=N_BLOCKS_FULL):
    """Build the per-core input dict. obs: [65536, 114] fp32."""
    NK = n_blocks * BK
    J = NK * NUM_ADV
    self18 = obs[:, :SELF_OBS_DIM]
    p = np.arange(J)
    b = p // BJ
    jl = p % BJ
    n = jl // BK
    kib = jl % BK
    r = NUM_ADV * (b * BK + kib) + n          # local mlp row (== global self row)
    kl = b * BK + kib                          # local batch row
    nbr = obs[NB_PER_CORE * core: NB_PER_CORE * core + NK,
              SELF_OBS_DIM:SELF_OBS_DIM + NUM_ADV * NBR_OBS_DIM]
    nbr = nbr.reshape(NK, NUM_ADV, NBR_OBS_DIM)
    feat = np.empty((J, 30), np.float32)
    feat[:, :18] = self18[r]
    feat[:, 18:] = nbr[kl, n]
    X = feat.reshape(n_blocks, 4, 512, 30).transpose(1, 3, 0, 2)  # [q, f, b, c]
    inp = np.zeros((4, 32, n_blocks, 512), np.float32)
    inp[:, :30] = X
    inp = inp.reshape(128, n_blocks * 512).astype(BF16)

    (eW1, eb1, eW2, eb2, vW1, vb1, vW2, vb2, aW1, ab1, aW2, ab2) = weights
    w1e = np.zeros((128, 128), np.float32)
    for q in range(4):
        w1e[32 * q:32 * q + 30] = eW1
    bias = np.stack([eb1, eb2, vb1, vb2, ab1, ab2], axis=1).astype(np.float32)
    ihd = np.zeros((128, 8), np.float32)
    for hd in range(8):
        ihd[16 * hd:16 * (hd + 1), hd] = 1.0
    return {
        "inp": inp,
        "w1e": w1e.astype(BF16),
        "w2e": eW2.astype(BF16),
        "w1v": vW1.astype(BF16),
        "w2v": vW2.astype(BF16),
        "w1a": aW1.astype(BF16),
        "w2a": aW2.astype(BF16),
        "bias": bias,
        "ones": np.ones((128, 1), BF16),
        "ihd": ihd.astype(BF16),
        "ident": np.eye(128, dtype=np.float32).astype(BF16),
        "ihdT": ihd.T.copy().astype(BF16),
    }


_NC_CACHE = {}


def _get_nc(n_blocks=N_BLOCKS_FULL):
    if n_blocks not in _NC_CACHE:
        _NC_CACHE[n_blocks] = build_bass(n_blocks)
    return _NC_CACHE[n_blocks]


def run_cores(obs, weights, n_blocks=N_BLOCKS_FULL, trace=False, **kw):
    nc = _get_nc(n_blocks)
    in_maps = [pack_core_inputs(obs, weights, d, n_blocks) for d in range(NCORES)]
    res = run_bass_kernel_spmd(nc, in_maps, core_ids=list(range(NCORES)),
                               trace=trace, **kw)
    return res


def kernel(obs, eW1, eb1, eW2, eb2, vW1, vb1, vW2, vb2, aW1, ab1, aW2, ab2,
           adv_obs_size=None, all_adv_obs_size=None, batch_size=None,
           num_groups=None, _trace=False, _res_out=None):
    obs = np.asarray(obs, dtype=np.float32)
    weights = tuple(np.asarray(w, dtype=np.float32)
                    for w in (eW1, eb1, eW2, eb2, vW1, vb1, vW2, vb2,
                              aW1, ab1, aW2, ab2))
    res = run_cores(obs, weights, trace=_trace)
    if _res_out is not None:
        _res_out.append(res)
    aa = np.empty((BATCH, HID), np.float32)
    out2 = np.empty((BATCH // NUM_AGENTS, HID), np.float32)
    for d in range(NCORES):
        aa[NB_PER_CORE * d:NB_PER_CORE * (d + 1)] = res.results[d]["aa"].T
        gd = NB_PER_CORE // NUM_AGENTS
        out2[gd * d:gd * (d + 1)] = res.results[d]["out2"].T
    multi_head = np.tile(out2, (NUM_AGENTS, 1))
    return multi_head, aa
